# revision 16
# baseline (speedup 1.0000x reference)
"""GNN message-passing layer on 8 TRN2 NeuronCores.

Reference computation (N=16384, D=128):
    a    = adj_mat.astype(f32)            # [N, N]
    deg  = a.sum(axis=0)                  # [N]
    agg  = (a^T @ x) / deg[:, None]       # [N, D]
    out  = relu(agg @ U^T)[None]          # [1, N, D]

Sharding: column-shard adj_mat across the 8 cores (core c owns output
nodes i in [c*2048, (c+1)*2048) and reads adj[:, islice]); x and U are
replicated. The contraction over j (all 16384 rows) is then fully local
to each core — no collective is needed, and each core emits its own
contiguous slice of the output.

Host-side staging (part of the sharding step, all value-lossless):
  - adj shard -> fp16 [16384, 2048]  ({0,1} exact; halves HBM traffic,
    which is the roofline term for this memory-bound problem)
  - x -> fp16 in [p, jb, d] layout so the device DMA is one contiguous
    transfer; U -> U^T fp16.

Per-core kernel:
  - adj shard is streamed in 128 row-blocks of [128, 2048] fp16.
  - aggT[d, i] = sum_j x[j, d] * a[j, i] accumulates in PSUM via
    matmul(lhsT=x_block [j,128d], rhs=a_block [j,512i]) — 4 psum banks
    of [128, 512] span the core's 2048 i values.
  - deg accumulates with a ones [j, 1] stationary vector. The four
    M=1 deg matmuls per block are packed into distinct 32-column PE
    groups (tile_position=(0, 32k)) so they execute concurrently on
    the 32x32 sub-arrays, costing ~1 matmul instead of 4. They share
    one PSUM bank at partitions {0, 32, 64, 96}.
  - finale: drain aggT to fp16 SBUF, transpose deg (4 x 512 rows ->
    [128, 16]) via a small DRAM bounce, reciprocal on DVE, then per
    128-i tile: h = matmul(lhsT=aggT[:, islice], rhs=U^T) -> psum
    [i, e] and relu(h * (1/deg_i)) fused into the psum->SBUF copy
    (ScalarE activation / DVE tensor_scalar, alternating). The output
    leaves as one [128, 16*128] DMA in [i_lo, t, e] layout which the
    host un-permutes.

fp16 is exact for the adjacency and deg; x/U rounding gives ~3e-4
relative error. All accumulation is fp32 in PSUM.
"""

import sys

if "/opt/trn_rl_repo" not in sys.path:
    sys.path.insert(0, "/opt/trn_rl_repo")

import numpy as np

from concourse import bacc, mybir, tile
from concourse.bass import ts
from concourse.bass_utils import run_bass_kernel_spmd

N = 16384  # nodes
D = 128  # features
CORES = 8
S = N // CORES  # 2048 output nodes per core
P = 128  # partitions
JB = N // P  # 128 row-blocks
IC = S // 512  # 4 moving-dim chunks of 512
T = S // P  # 16 output tiles per core

F16 = mybir.dt.float16
F32 = mybir.dt.float32
F8 = mybir.dt.float8e4


def build_nc():
    nc = bacc.Bacc("TRN2", target_bir_lowering=False, debug=False)

    a_dram = nc.dram_tensor("a", [N, S], F8, kind="ExternalInput").ap()
    x_dram = nc.dram_tensor("x", [P, JB * D], F16, kind="ExternalInput").ap()
    ut_dram = nc.dram_tensor("ut", [D, D], F16, kind="ExternalInput").ap()
    # [i_lo, t, e] layout; host un-permutes to [2048, 128]
    out_dram = nc.dram_tensor("out", [P, T * D], F32, kind="ExternalOutput").ap()

    with tile.TileContext(nc) as tc:
        CH = 8  # row-blocks per adj DMA chunk (2 MB fp8 transfers)
        with (
            tc.tile_pool(name="persist", bufs=1) as persist,
            tc.tile_pool(name="adj", bufs=3) as adj_pool,
            tc.tile_pool(name="dram", bufs=1, space="DRAM") as dram_pool,
        ):
            xh = persist.tile([P, JB, D], F16)
            ut16 = persist.tile([D, D], F16)
            # fp8 ones for the DoubleRow deg matmuls: [K, 2, 16] so the
            # middle (row-pair) dim has a 16-aligned element step
            ones8 = persist.tile([P, 2, 16], F8)
            nc.gpsimd.memset(ones8[:], 1.0)

            ag16 = persist.tile([P, S], F16)
            deg_sb = persist.tile([P, 512], F32)  # rows {0,32,64,96} hold deg
            degT = persist.tile([P, T], F32)
            rdeg = persist.tile([P, T], F32)

            with tc.tile_pool(name="mmps", bufs=1, space="PSUM") as mmps:
                ps_agg = [mmps.tile([P, 512], F32, name=f"ps_agg{i}") for i in range(IC)]
                ps_deg = mmps.tile([P, 512], F32, name="ps_deg")

                x_r = x_dram.rearrange("p (g jb d) -> p g jb d", g=4, jb=JB // 4)
                for ck in range(JB // CH):
                    af = adj_pool.tile([P, CH, S], F8, tag="af")
                    # alternate the two HWDGE rings; keep ring 1 (scalar)
                    # busy with the x/ut prologue during the first chunks
                    eng = nc.sync if ck % 2 == 0 else nc.scalar
                    nc_src = a_dram[ck * CH * P : (ck + 1) * CH * P, :]
                    eng.dma_start(af[:], nc_src.rearrange("(c p) i -> p c i", p=P))
                    # interleave the x prologue in 1 MB chunks on ring 1 so
                    # the first x chunk (gating the first matmul) lands fast
                    if ck < 4:
                        g = ck
                        nc.scalar.dma_start(
                            xh[:, ts(g, JB // 4), :], x_r[:, g, :, :]
                        )
                        if ck == 0:
                            nc.scalar.dma_start(ut16[:], ut_dram[:])
                    for c in range(CH):
                        jb = ck * CH + c
                        first, last = jb == 0, jb == JB - 1
                        for ic in range(IC):
                            nc.tensor.matmul(
                                ps_agg[ic][:],
                                xh[:, jb, :],
                                af[:, c, ts(ic, 512)],
                                start=first,
                                stop=last,
                            )
                        for ic in range(IC):
                            nc.tensor.matmul(
                                ps_deg[32 * ic : 32 * ic + 1, :],
                                ones8[:, 0, 0:1],
                                af[:, c, ts(ic, 512)],
                                start=first,
                                stop=last,
                                tile_position=(0, 32 * ic),
                            )

                # drain deg rows first: the DRAM bounce + reciprocal chain
                # gates the finale (DVE requires partition step 1, so 4 ops)
                for ic in range(IC):
                    nc.vector.tensor_copy(
                        deg_sb[32 * ic : 32 * ic + 1, :],
                        ps_deg[32 * ic : 32 * ic + 1, :],
                    )
                for ic in range(IC):
                    eng = nc.vector if ic % 2 == 0 else nc.scalar
                    if ic % 2 == 0:
                        nc.vector.tensor_copy(ag16[:, ts(ic, 512)], ps_agg[ic][:])
                    else:
                        nc.scalar.copy(ag16[:, ts(ic, 512)], ps_agg[ic][:])

            # transpose deg -> [128, 16] via DRAM bounce
            deg_dram = dram_pool.tile([IC, 512], F32)
            for ic in range(IC):
                nc.scalar.dma_start(
                    deg_dram[ic : ic + 1, :], deg_sb[32 * ic : 32 * ic + 1, :]
                )
            nc.scalar.dma_start(
                degT[:], deg_dram.rearrange("a b -> (a b)").rearrange("(t p) -> p t", p=P)
            )
            nc.vector.reciprocal(rdeg[:], degT[:])

            o_all = persist.tile([P, T, D], F32)
            with tc.tile_pool(name="fps", bufs=3, space="PSUM") as fps:
                for t in range(T):
                    h_ps = fps.tile([P, D], F32, tag="h")
                    nc.tensor.matmul(
                        h_ps[:], ag16[:, ts(t, P)], ut16[:], start=True, stop=True
                    )
                    if t % 2 == 0:
                        # ScalarE: out = relu(h * rdeg)
                        nc.scalar.activation(
                            o_all[:, t, :],
                            h_ps[:],
                            mybir.ActivationFunctionType.Relu,
                            scale=rdeg[:, t : t + 1],
                        )
                    else:
                        # DVE: out = max(h * rdeg, 0)
                        nc.vector.tensor_scalar(
                            o_all[:, t, :],
                            h_ps[:],
                            rdeg[:, t : t + 1],
                            0.0,
                            mybir.AluOpType.mult,
                            mybir.AluOpType.max,
                        )
            nc.sync.dma_start(
                out_dram.rearrange("p (t d) -> p t d", t=T), o_all[:]
            )

    nc.compile()
    return nc


_NC = None


def _get_nc():
    global _NC
    if _NC is None:
        _NC = build_nc()
    return _NC


def prep_in_maps(x, adj_mat, U):
    import ml_dtypes

    x = np.asarray(x, dtype=np.float32)
    adj_mat = np.asarray(adj_mat)
    U = np.asarray(U, dtype=np.float32)
    # x -> fp16 [p, jb, d] flattened to [128, JB*D]
    xt = np.ascontiguousarray(
        x.reshape(JB, P, D).transpose(1, 0, 2).astype(np.float16).reshape(P, JB * D)
    )
    ut = np.ascontiguousarray(U.T.astype(np.float16))
    # adjacency values are {0,1}: exact in fp8e4m3, and the int8 bit
    # patterns 0x00/0x38 can be produced by a table lookup (much faster
    # than a float astype over 1 GiB)
    lut = np.zeros(2, dtype=np.uint8)
    lut[1] = np.array(1.0, dtype=ml_dtypes.float8_e4m3).view(np.uint8)
    in_maps = []
    for c in range(CORES):
        a8 = lut[adj_mat[:, c * S : (c + 1) * S]].view(ml_dtypes.float8_e4m3)
        in_maps.append({"a": a8, "x": xt, "ut": ut})
    return in_maps


def assemble_out(results):
    # per-core out is [128, T*D] in [i_lo, t, e] layout
    parts = []
    for c in range(CORES):
        o = results[c]["out"].reshape(P, T, D).transpose(1, 0, 2).reshape(S, D)
        parts.append(o)
    return np.concatenate(parts, axis=0)[None]


def kernel(x, adj_mat, U, **_):
    nc = _get_nc()
    in_maps = prep_in_maps(x, adj_mat, U)
    res = run_bass_kernel_spmd(nc, in_maps, core_ids=list(range(CORES)))
    return assemble_out(res.results)


# revision 22
# speedup vs baseline: 1.0018x; 1.0018x over previous
"""GNN message-passing layer on 8 TRN2 NeuronCores.

Reference computation (N=16384, D=128):
    a    = adj_mat.astype(f32)            # [N, N]
    deg  = a.sum(axis=0)                  # [N]
    agg  = (a^T @ x) / deg[:, None]       # [N, D]
    out  = relu(agg @ U^T)[None]          # [1, N, D]

Sharding: column-shard adj_mat across the 8 cores (core c owns output
nodes i in [c*2048, (c+1)*2048) and reads adj[:, islice]); x and U are
replicated. The contraction over j (all 16384 rows) is then fully local
to each core — no collective is needed, and each core emits its own
contiguous slice of the output.

Host-side staging (part of the sharding step, all value-lossless):
  - adj shard -> fp16 [16384, 2048]  ({0,1} exact; halves HBM traffic,
    which is the roofline term for this memory-bound problem)
  - x -> fp16 in [p, jb, d] layout so the device DMA is one contiguous
    transfer; U -> U^T fp16.

Per-core kernel:
  - adj shard is streamed in 128 row-blocks of [128, 2048] fp16.
  - aggT[d, i] = sum_j x[j, d] * a[j, i] accumulates in PSUM via
    matmul(lhsT=x_block [j,128d], rhs=a_block [j,512i]) — 4 psum banks
    of [128, 512] span the core's 2048 i values.
  - deg accumulates with a ones [j, 1] stationary vector. The four
    M=1 deg matmuls per block are packed into distinct 32-column PE
    groups (tile_position=(0, 32k)) so they execute concurrently on
    the 32x32 sub-arrays, costing ~1 matmul instead of 4. They share
    one PSUM bank at partitions {0, 32, 64, 96}.
  - finale: drain aggT to fp16 SBUF, transpose deg (4 x 512 rows ->
    [128, 16]) via a small DRAM bounce, reciprocal on DVE, then per
    128-i tile: h = matmul(lhsT=aggT[:, islice], rhs=U^T) -> psum
    [i, e] and relu(h * (1/deg_i)) fused into the psum->SBUF copy
    (ScalarE activation / DVE tensor_scalar, alternating). The output
    leaves as one [128, 16*128] DMA in [i_lo, t, e] layout which the
    host un-permutes.

fp16 is exact for the adjacency and deg; x/U rounding gives ~3e-4
relative error. All accumulation is fp32 in PSUM.
"""

import sys

if "/opt/trn_rl_repo" not in sys.path:
    sys.path.insert(0, "/opt/trn_rl_repo")

import numpy as np

from concourse import bacc, mybir, tile
from concourse.bass import ts
from concourse.bass_utils import run_bass_kernel_spmd

N = 16384  # nodes
D = 128  # features
CORES = 8
S = N // CORES  # 2048 output nodes per core
P = 128  # partitions
JB = N // P  # 128 row-blocks
IC = S // 512  # 4 moving-dim chunks of 512
T = S // P  # 16 output tiles per core

F16 = mybir.dt.float16
F32 = mybir.dt.float32
F8 = mybir.dt.float8e4


def build_nc():
    nc = bacc.Bacc("TRN2", target_bir_lowering=False, debug=False)

    a_dram = nc.dram_tensor("a", [N, S], F8, kind="ExternalInput").ap()
    x_dram = nc.dram_tensor("x", [P, JB * D], F16, kind="ExternalInput").ap()
    ut_dram = nc.dram_tensor("ut", [D, D], F16, kind="ExternalInput").ap()
    # [i_lo, t, e] layout; host un-permutes to [2048, 128]
    out_dram = nc.dram_tensor("out", [P, T * D], F32, kind="ExternalOutput").ap()

    with tile.TileContext(nc) as tc:
        CH = 8  # row-blocks per adj DMA chunk (2 MB fp8 transfers)
        with (
            tc.tile_pool(name="persist", bufs=1) as persist,
            tc.tile_pool(name="adj", bufs=3) as adj_pool,
            tc.tile_pool(name="dram", bufs=1, space="DRAM") as dram_pool,
        ):
            xh = persist.tile([P, JB, D], F16)
            ut16 = persist.tile([D, D], F16)
            # fp8 ones for the DoubleRow deg matmuls: [K, 2, 16] so the
            # middle (row-pair) dim has a 16-aligned element step
            ones8 = persist.tile([P, 2, 16], F8)
            nc.gpsimd.memset(ones8[:], 1.0)

            ag16 = persist.tile([P, S], F16)
            deg_sb = persist.tile([P, 512], F32)  # rows {0,32,64,96} hold deg
            degT = persist.tile([P, T], F32)
            rdeg = persist.tile([P, T], F32)

            with tc.tile_pool(name="mmps", bufs=1, space="PSUM") as mmps:
                ps_agg = [mmps.tile([P, 512], F32, name=f"ps_agg{i}") for i in range(IC)]
                ps_deg = mmps.tile([P, 512], F32, name="ps_deg")

                x_r = x_dram.rearrange("p (g jb d) -> p g jb d", g=4, jb=JB // 4)
                for ck in range(JB // CH):
                    af = adj_pool.tile([P, CH, S], F8, tag="af")
                    # alternate the two HWDGE rings; keep ring 1 (scalar)
                    # busy with the x/ut prologue during the first chunks
                    eng = nc.sync if ck % 2 == 0 else nc.scalar
                    nc_src = a_dram[ck * CH * P : (ck + 1) * CH * P, :]
                    src_r = nc_src.rearrange("(c p) i -> p c i", p=P)
                    if ck == 0:
                        # split the first chunk so the opening matmuls are
                        # not gated on a full 2 MB transfer
                        eng.dma_start(af[:, 0 : CH // 4, :], src_r[:, 0 : CH // 4, :])
                        eng.dma_start(af[:, CH // 4 :, :], src_r[:, CH // 4 :, :])
                    else:
                        eng.dma_start(af[:], src_r)
                    # interleave the x prologue in 1 MB chunks on ring 1 so
                    # the first x chunk (gating the first matmul) lands fast
                    if ck < 4:
                        g = ck
                        nc.scalar.dma_start(
                            xh[:, ts(g, JB // 4), :], x_r[:, g, :, :]
                        )
                        if ck == 0:
                            nc.scalar.dma_start(ut16[:], ut_dram[:])
                    for c in range(CH):
                        jb = ck * CH + c
                        first, last = jb == 0, jb == JB - 1
                        for ic in range(IC):
                            nc.tensor.matmul(
                                ps_agg[ic][:],
                                xh[:, jb, :],
                                af[:, c, ts(ic, 512)],
                                start=first,
                                stop=last,
                            )
                        for ic in range(IC):
                            nc.tensor.matmul(
                                ps_deg[32 * ic : 32 * ic + 1, :],
                                ones8[:, 0, 0:1],
                                af[:, c, ts(ic, 512)],
                                start=first,
                                stop=last,
                                tile_position=(0, 32 * ic),
                            )

                # drain deg rows first: the DRAM bounce + reciprocal chain
                # gates the finale (DVE requires partition step 1, so 4 ops)
                for ic in range(IC):
                    nc.vector.tensor_copy(
                        deg_sb[32 * ic : 32 * ic + 1, :],
                        ps_deg[32 * ic : 32 * ic + 1, :],
                    )
                for ic in range(IC):
                    eng = nc.vector if ic % 2 == 0 else nc.scalar
                    if ic % 2 == 0:
                        nc.vector.tensor_copy(ag16[:, ts(ic, 512)], ps_agg[ic][:])
                    else:
                        nc.scalar.copy(ag16[:, ts(ic, 512)], ps_agg[ic][:])

            # transpose deg -> [128, 16] via DRAM bounce
            deg_dram = dram_pool.tile([IC, 512], F32)
            for ic in range(IC):
                nc.scalar.dma_start(
                    deg_dram[ic : ic + 1, :], deg_sb[32 * ic : 32 * ic + 1, :]
                )
            nc.scalar.dma_start(
                degT[:], deg_dram.rearrange("a b -> (a b)").rearrange("(t p) -> p t", p=P)
            )
            nc.vector.reciprocal(rdeg[:], degT[:])

            # two output halves in separate tiles so the first half's DMA
            # leaves while the second half is still computing
            o_halves = [
                persist.tile([P, T // 2, D], F32, name=f"o_half{h}") for h in range(2)
            ]
            out_r = out_dram.rearrange("p (t d) -> p t d", t=T)
            with tc.tile_pool(name="fps", bufs=3, space="PSUM") as fps:
                for t in range(T):
                    h_ps = fps.tile([P, D], F32, tag="h")
                    nc.tensor.matmul(
                        h_ps[:], ag16[:, ts(t, P)], ut16[:], start=True, stop=True
                    )
                    o_dst = o_halves[t // (T // 2)][:, t % (T // 2), :]
                    if t % 2 == 0:
                        # ScalarE: out = relu(h * rdeg)
                        nc.scalar.activation(
                            o_dst,
                            h_ps[:],
                            mybir.ActivationFunctionType.Relu,
                            scale=rdeg[:, t : t + 1],
                        )
                    else:
                        # DVE: out = max(h * rdeg, 0)
                        nc.vector.tensor_scalar(
                            o_dst,
                            h_ps[:],
                            rdeg[:, t : t + 1],
                            0.0,
                            mybir.AluOpType.mult,
                            mybir.AluOpType.max,
                        )
                nc.scalar.dma_start(out_r[:, 0 : T // 2, :], o_halves[0][:])
                nc.sync.dma_start(out_r[:, T // 2 : T, :], o_halves[1][:])

    nc.compile()
    return nc


_NC = None


def _get_nc():
    global _NC
    if _NC is None:
        _NC = build_nc()
    return _NC


def prep_in_maps(x, adj_mat, U):
    import ml_dtypes

    x = np.asarray(x, dtype=np.float32)
    adj_mat = np.asarray(adj_mat)
    U = np.asarray(U, dtype=np.float32)
    # x -> fp16 [p, jb, d] flattened to [128, JB*D]
    xt = np.ascontiguousarray(
        x.reshape(JB, P, D).transpose(1, 0, 2).astype(np.float16).reshape(P, JB * D)
    )
    ut = np.ascontiguousarray(U.T.astype(np.float16))
    # adjacency values are {0,1}: exact in fp8e4m3, and the int8 bit
    # patterns 0x00/0x38 can be produced by a table lookup (much faster
    # than a float astype over 1 GiB)
    lut = np.zeros(2, dtype=np.uint8)
    lut[1] = np.array(1.0, dtype=ml_dtypes.float8_e4m3).view(np.uint8)
    in_maps = []
    for c in range(CORES):
        a8 = lut[adj_mat[:, c * S : (c + 1) * S]].view(ml_dtypes.float8_e4m3)
        in_maps.append({"a": a8, "x": xt, "ut": ut})
    return in_maps


def assemble_out(results):
    # per-core out is [128, T*D] in [i_lo, t, e] layout
    parts = []
    for c in range(CORES):
        o = results[c]["out"].reshape(P, T, D).transpose(1, 0, 2).reshape(S, D)
        parts.append(o)
    return np.concatenate(parts, axis=0)[None]


def kernel(x, adj_mat, U, **_):
    nc = _get_nc()
    in_maps = prep_in_maps(x, adj_mat, U)
    res = run_bass_kernel_spmd(nc, in_maps, core_ids=list(range(CORES)))
    return assemble_out(res.results)


# revision 24
# speedup vs baseline: 1.0114x; 1.0096x over previous
"""GNN message-passing layer on 8 TRN2 NeuronCores.

Reference computation (N=16384, D=128):
    a    = adj_mat.astype(f32)            # [N, N]
    deg  = a.sum(axis=0)                  # [N]
    agg  = (a^T @ x) / deg[:, None]       # [N, D]
    out  = relu(agg @ U^T)[None]          # [1, N, D]

Sharding: column-shard adj_mat across the 8 cores (core c owns output
nodes i in [c*2048, (c+1)*2048) and reads adj[:, islice]); x and U are
replicated. The contraction over j (all 16384 rows) is then fully local
to each core — no collective is needed, and each core emits its own
contiguous slice of the output.

Host-side staging (part of the sharding step, all value-lossless):
  - adj shard -> fp16 [16384, 2048]  ({0,1} exact; halves HBM traffic,
    which is the roofline term for this memory-bound problem)
  - x -> fp16 in [p, jb, d] layout so the device DMA is one contiguous
    transfer; U -> U^T fp16.

Per-core kernel:
  - adj shard is streamed in 128 row-blocks of [128, 2048] fp16.
  - aggT[d, i] = sum_j x[j, d] * a[j, i] accumulates in PSUM via
    matmul(lhsT=x_block [j,128d], rhs=a_block [j,512i]) — 4 psum banks
    of [128, 512] span the core's 2048 i values.
  - deg accumulates with a ones [j, 1] stationary vector. The four
    M=1 deg matmuls per block are packed into distinct 32-column PE
    groups (tile_position=(0, 32k)) so they execute concurrently on
    the 32x32 sub-arrays, costing ~1 matmul instead of 4. They share
    one PSUM bank at partitions {0, 32, 64, 96}.
  - finale: drain aggT to fp16 SBUF, transpose deg (4 x 512 rows ->
    [128, 16]) via a small DRAM bounce, reciprocal on DVE, then per
    128-i tile: h = matmul(lhsT=aggT[:, islice], rhs=U^T) -> psum
    [i, e] and relu(h * (1/deg_i)) fused into the psum->SBUF copy
    (ScalarE activation / DVE tensor_scalar, alternating). The output
    leaves as one [128, 16*128] DMA in [i_lo, t, e] layout which the
    host un-permutes.

fp16 is exact for the adjacency and deg; x/U rounding gives ~3e-4
relative error. All accumulation is fp32 in PSUM.
"""

import sys

if "/opt/trn_rl_repo" not in sys.path:
    sys.path.insert(0, "/opt/trn_rl_repo")

import numpy as np

from concourse import bacc, mybir, tile
from concourse.bass import ts
from concourse.bass_utils import run_bass_kernel_spmd

N = 16384  # nodes
D = 128  # features
CORES = 8
S = N // CORES  # 2048 output nodes per core
P = 128  # partitions
JB = N // P  # 128 row-blocks
IC = S // 512  # 4 moving-dim chunks of 512
T = S // P  # 16 output tiles per core

F16 = mybir.dt.float16
F32 = mybir.dt.float32
F8 = mybir.dt.float8e4


def build_nc():
    nc = bacc.Bacc("TRN2", target_bir_lowering=False, debug=False)

    a_dram = nc.dram_tensor("a", [N, S], F8, kind="ExternalInput").ap()
    x_dram = nc.dram_tensor("x", [P, JB * D], F16, kind="ExternalInput").ap()
    ut_dram = nc.dram_tensor("ut", [D, D], F16, kind="ExternalInput").ap()
    # [i_lo, t, e] layout; host un-permutes to [2048, 128]
    out_dram = nc.dram_tensor("out", [P, T * D], F32, kind="ExternalOutput").ap()

    with tile.TileContext(nc) as tc:
        CH = 8  # row-blocks per adj DMA chunk (2 MB fp8 transfers)
        with (
            tc.tile_pool(name="persist", bufs=1) as persist,
            tc.tile_pool(name="adj", bufs=4) as adj_pool,
            tc.tile_pool(name="dram", bufs=1, space="DRAM") as dram_pool,
        ):
            xh = persist.tile([P, JB, D], F16)
            ut16 = persist.tile([D, D], F16)
            # fp8 ones for the DoubleRow deg matmuls: [K, 2, 16] so the
            # middle (row-pair) dim has a 16-aligned element step
            ones8 = persist.tile([P, 2, 16], F8)
            nc.gpsimd.memset(ones8[:], 1.0)

            ag16 = persist.tile([P, S], F16)
            deg_sb = persist.tile([P, 512], F32)  # rows {0,32,64,96} hold deg
            degT = persist.tile([P, T], F32)
            rdeg = persist.tile([P, T], F32)

            with tc.tile_pool(name="mmps", bufs=1, space="PSUM") as mmps:
                ps_agg = [mmps.tile([P, 512], F32, name=f"ps_agg{i}") for i in range(IC)]
                ps_deg = mmps.tile([P, 512], F32, name="ps_deg")

                x_r = x_dram.rearrange("p (g jb d) -> p g jb d", g=4, jb=JB // 4)
                for ck in range(JB // CH):
                    af = adj_pool.tile([P, CH, S], F8, tag="af")
                    # alternate the two HWDGE rings; keep ring 1 (scalar)
                    # busy with the x/ut prologue during the first chunks
                    eng = nc.sync if ck % 2 == 0 else nc.scalar
                    nc_src = a_dram[ck * CH * P : (ck + 1) * CH * P, :]
                    src_r = nc_src.rearrange("(c p) i -> p c i", p=P)
                    if ck == 0:
                        # split the first chunk so the opening matmuls are
                        # not gated on a full 2 MB transfer
                        eng.dma_start(af[:, 0 : CH // 4, :], src_r[:, 0 : CH // 4, :])
                        eng.dma_start(af[:, CH // 4 :, :], src_r[:, CH // 4 :, :])
                    else:
                        eng.dma_start(af[:], src_r)
                    # x prologue in 1 MB chunks on ring 1: chunk g gates the
                    # matmuls from row-block 32*g, so only chunk 0 must land
                    # early — spreading the rest keeps the opening adj
                    # transfers from queuing behind them on the SDMA engines
                    if ck in (0, 3, 7, 11):
                        g = (0, 3, 7, 11).index(ck)
                        nc.scalar.dma_start(
                            xh[:, ts(g, JB // 4), :], x_r[:, g, :, :]
                        )
                        if ck == 0:
                            nc.scalar.dma_start(ut16[:], ut_dram[:])
                    for c in range(CH):
                        jb = ck * CH + c
                        first, last = jb == 0, jb == JB - 1
                        for ic in range(IC):
                            nc.tensor.matmul(
                                ps_agg[ic][:],
                                xh[:, jb, :],
                                af[:, c, ts(ic, 512)],
                                start=first,
                                stop=last,
                            )
                        for ic in range(IC):
                            nc.tensor.matmul(
                                ps_deg[32 * ic : 32 * ic + 1, :],
                                ones8[:, 0, 0:1],
                                af[:, c, ts(ic, 512)],
                                start=first,
                                stop=last,
                                tile_position=(0, 32 * ic),
                            )

                # drain deg rows first: the DRAM bounce + reciprocal chain
                # gates the finale (DVE requires partition step 1, so 4 ops)
                for ic in range(IC):
                    nc.vector.tensor_copy(
                        deg_sb[32 * ic : 32 * ic + 1, :],
                        ps_deg[32 * ic : 32 * ic + 1, :],
                    )
                for ic in range(IC):
                    eng = nc.vector if ic % 2 == 0 else nc.scalar
                    if ic % 2 == 0:
                        nc.vector.tensor_copy(ag16[:, ts(ic, 512)], ps_agg[ic][:])
                    else:
                        nc.scalar.copy(ag16[:, ts(ic, 512)], ps_agg[ic][:])

            # transpose deg -> [128, 16] via DRAM bounce
            deg_dram = dram_pool.tile([IC, 512], F32)
            for ic in range(IC):
                nc.scalar.dma_start(
                    deg_dram[ic : ic + 1, :], deg_sb[32 * ic : 32 * ic + 1, :]
                )
            nc.scalar.dma_start(
                degT[:], deg_dram.rearrange("a b -> (a b)").rearrange("(t p) -> p t", p=P)
            )
            nc.vector.reciprocal(rdeg[:], degT[:])

            # two output halves in separate tiles so the first half's DMA
            # leaves while the second half is still computing
            o_halves = [
                persist.tile([P, T // 2, D], F32, name=f"o_half{h}") for h in range(2)
            ]
            out_r = out_dram.rearrange("p (t d) -> p t d", t=T)
            with tc.tile_pool(name="fps", bufs=3, space="PSUM") as fps:
                for t in range(T):
                    h_ps = fps.tile([P, D], F32, tag="h")
                    nc.tensor.matmul(
                        h_ps[:], ag16[:, ts(t, P)], ut16[:], start=True, stop=True
                    )
                    o_dst = o_halves[t // (T // 2)][:, t % (T // 2), :]
                    if t % 2 == 0:
                        # ScalarE: out = relu(h * rdeg)
                        nc.scalar.activation(
                            o_dst,
                            h_ps[:],
                            mybir.ActivationFunctionType.Relu,
                            scale=rdeg[:, t : t + 1],
                        )
                    else:
                        # DVE: out = max(h * rdeg, 0)
                        nc.vector.tensor_scalar(
                            o_dst,
                            h_ps[:],
                            rdeg[:, t : t + 1],
                            0.0,
                            mybir.AluOpType.mult,
                            mybir.AluOpType.max,
                        )
                nc.scalar.dma_start(out_r[:, 0 : T // 2, :], o_halves[0][:])
                nc.sync.dma_start(out_r[:, T // 2 : T, :], o_halves[1][:])

    nc.compile()
    return nc


_NC = None


def _get_nc():
    global _NC
    if _NC is None:
        _NC = build_nc()
    return _NC


def prep_in_maps(x, adj_mat, U):
    import ml_dtypes

    x = np.asarray(x, dtype=np.float32)
    adj_mat = np.asarray(adj_mat)
    U = np.asarray(U, dtype=np.float32)
    # x -> fp16 [p, jb, d] flattened to [128, JB*D]
    xt = np.ascontiguousarray(
        x.reshape(JB, P, D).transpose(1, 0, 2).astype(np.float16).reshape(P, JB * D)
    )
    ut = np.ascontiguousarray(U.T.astype(np.float16))
    # adjacency values are {0,1}: exact in fp8e4m3, and the int8 bit
    # patterns 0x00/0x38 can be produced by a table lookup (much faster
    # than a float astype over 1 GiB)
    lut = np.zeros(2, dtype=np.uint8)
    lut[1] = np.array(1.0, dtype=ml_dtypes.float8_e4m3).view(np.uint8)
    in_maps = []
    for c in range(CORES):
        a8 = lut[adj_mat[:, c * S : (c + 1) * S]].view(ml_dtypes.float8_e4m3)
        in_maps.append({"a": a8, "x": xt, "ut": ut})
    return in_maps


def assemble_out(results):
    # per-core out is [128, T*D] in [i_lo, t, e] layout
    parts = []
    for c in range(CORES):
        o = results[c]["out"].reshape(P, T, D).transpose(1, 0, 2).reshape(S, D)
        parts.append(o)
    return np.concatenate(parts, axis=0)[None]


def kernel(x, adj_mat, U, **_):
    nc = _get_nc()
    in_maps = prep_in_maps(x, adj_mat, U)
    res = run_bass_kernel_spmd(nc, in_maps, core_ids=list(range(CORES)))
    return assemble_out(res.results)


# revision 26
# speedup vs baseline: 1.0240x; 1.0124x over previous
"""GNN message-passing layer on 8 TRN2 NeuronCores.

Reference computation (N=16384, D=128):
    a    = adj_mat.astype(f32)            # [N, N]
    deg  = a.sum(axis=0)                  # [N]
    agg  = (a^T @ x) / deg[:, None]       # [N, D]
    out  = relu(agg @ U^T)[None]          # [1, N, D]

Sharding: column-shard adj_mat across the 8 cores (core c owns output
nodes i in [c*2048, (c+1)*2048) and reads adj[:, islice]); x and U are
replicated. The contraction over j (all 16384 rows) is then fully local
to each core — no collective is needed, and each core emits its own
contiguous slice of the output.

Host-side staging (part of the sharding step, all value-lossless):
  - adj shard -> fp16 [16384, 2048]  ({0,1} exact; halves HBM traffic,
    which is the roofline term for this memory-bound problem)
  - x -> fp16 in [p, jb, d] layout so the device DMA is one contiguous
    transfer; U -> U^T fp16.

Per-core kernel:
  - adj shard is streamed in 128 row-blocks of [128, 2048] fp16.
  - aggT[d, i] = sum_j x[j, d] * a[j, i] accumulates in PSUM via
    matmul(lhsT=x_block [j,128d], rhs=a_block [j,512i]) — 4 psum banks
    of [128, 512] span the core's 2048 i values.
  - deg accumulates with a ones [j, 1] stationary vector. The four
    M=1 deg matmuls per block are packed into distinct 32-column PE
    groups (tile_position=(0, 32k)) so they execute concurrently on
    the 32x32 sub-arrays, costing ~1 matmul instead of 4. They share
    one PSUM bank at partitions {0, 32, 64, 96}.
  - finale: drain aggT to fp16 SBUF, transpose deg (4 x 512 rows ->
    [128, 16]) via a small DRAM bounce, reciprocal on DVE, then per
    128-i tile: h = matmul(lhsT=aggT[:, islice], rhs=U^T) -> psum
    [i, e] and relu(h * (1/deg_i)) fused into the psum->SBUF copy
    (ScalarE activation / DVE tensor_scalar, alternating). The output
    leaves as one [128, 16*128] DMA in [i_lo, t, e] layout which the
    host un-permutes.

fp16 is exact for the adjacency and deg; x/U rounding gives ~3e-4
relative error. All accumulation is fp32 in PSUM.
"""

import sys

if "/opt/trn_rl_repo" not in sys.path:
    sys.path.insert(0, "/opt/trn_rl_repo")

import numpy as np

from concourse import bacc, mybir, tile
from concourse.bass import ts
from concourse.bass_utils import run_bass_kernel_spmd

N = 16384  # nodes
D = 128  # features
CORES = 8
S = N // CORES  # 2048 output nodes per core
P = 128  # partitions
JB = N // P  # 128 row-blocks
IC = S // 512  # 4 moving-dim chunks of 512
T = S // P  # 16 output tiles per core

F16 = mybir.dt.float16
F32 = mybir.dt.float32
F8 = mybir.dt.float8e4


def build_nc():
    nc = bacc.Bacc("TRN2", target_bir_lowering=False, debug=False)

    a_dram = nc.dram_tensor("a", [N, S], F8, kind="ExternalInput").ap()
    x_dram = nc.dram_tensor("x", [P, JB * D], F16, kind="ExternalInput").ap()
    ut_dram = nc.dram_tensor("ut", [D, D], F16, kind="ExternalInput").ap()
    # [i_lo, t, e] layout; host un-permutes to [2048, 128]
    out_dram = nc.dram_tensor("out", [P, T * D], F32, kind="ExternalOutput").ap()

    with tile.TileContext(nc) as tc:
        CH = 8  # row-blocks per adj DMA chunk (2 MB fp8 transfers)
        with (
            tc.tile_pool(name="persist", bufs=1) as persist,
            tc.tile_pool(name="adj", bufs=4) as adj_pool,
            tc.tile_pool(name="dram", bufs=1, space="DRAM") as dram_pool,
        ):
            xh = persist.tile([P, JB, D], F16)
            ut16 = persist.tile([D, D], F16)
            # fp8 ones for the DoubleRow deg matmuls: [K, 2, 16] so the
            # middle (row-pair) dim has a 16-aligned element step
            ones8 = persist.tile([P, 2, 16], F8)
            nc.gpsimd.memset(ones8[:], 1.0)

            ag16 = persist.tile([P, S], F16)
            deg_sb = persist.tile([P, 512], F32)  # rows {0,32,64,96} hold deg
            degT = persist.tile([P, T], F32)
            rdeg = persist.tile([P, T], F32)

            with tc.tile_pool(name="mmps", bufs=1, space="PSUM") as mmps:
                ps_agg = [mmps.tile([P, 512], F32, name=f"ps_agg{i}") for i in range(IC)]
                ps_deg = mmps.tile([P, 512], F32, name="ps_deg")

                x_r = x_dram.rearrange("p (g jb d) -> p g jb d", g=4, jb=JB // 4)
                for ck in range(JB // CH):
                    af = adj_pool.tile([P, CH, S], F8, tag="af")
                    # alternate the two HWDGE rings; keep ring 1 (scalar)
                    # busy with the x/ut prologue during the first chunks
                    eng = nc.sync if ck % 2 == 0 else nc.scalar
                    nc_src = a_dram[ck * CH * P : (ck + 1) * CH * P, :]
                    src_r = nc_src.rearrange("(c p) i -> p c i", p=P)
                    if ck == 0:
                        # split the first chunk so the opening matmuls are
                        # not gated on a full 2 MB transfer
                        eng.dma_start(af[:, 0 : CH // 4, :], src_r[:, 0 : CH // 4, :])
                        eng.dma_start(af[:, CH // 4 :, :], src_r[:, CH // 4 :, :])
                    else:
                        eng.dma_start(af[:], src_r)
                    # x prologue in 1 MB chunks on ring 1: chunk g gates the
                    # matmuls from row-block 32*g, so only chunk 0 must land
                    # early — spreading the rest keeps the opening adj
                    # transfers from queuing behind them on the SDMA engines
                    if ck in (0, 3, 7, 11):
                        g = (0, 3, 7, 11).index(ck)
                        nc.scalar.dma_start(
                            xh[:, ts(g, JB // 4), :], x_r[:, g, :, :]
                        )
                        if ck == 0:
                            nc.scalar.dma_start(ut16[:], ut_dram[:])
                    for c in range(CH):
                        jb = ck * CH + c
                        first, last = jb == 0, jb == JB - 1
                        for ic in range(IC):
                            nc.tensor.matmul(
                                ps_agg[ic][:],
                                xh[:, jb, :],
                                af[:, c, ts(ic, 512)],
                                start=first,
                                stop=last,
                            )
                        for ic in range(IC):
                            nc.tensor.matmul(
                                ps_deg[32 * ic : 32 * ic + 1, :],
                                ones8[:, 0, 0:1],
                                af[:, c, ts(ic, 512)],
                                start=first,
                                stop=last,
                                tile_position=(0, 32 * ic),
                            )

                # drain deg rows first: the DRAM bounce + reciprocal chain
                # gates the finale (DVE requires partition step 1, so 4 ops)
                for ic in range(IC):
                    nc.vector.tensor_copy(
                        deg_sb[32 * ic : 32 * ic + 1, :],
                        ps_deg[32 * ic : 32 * ic + 1, :],
                    )
                for ic in range(IC):
                    eng = nc.vector if ic % 2 == 0 else nc.scalar
                    if ic % 2 == 0:
                        nc.vector.tensor_copy(ag16[:, ts(ic, 512)], ps_agg[ic][:])
                    else:
                        nc.scalar.copy(ag16[:, ts(ic, 512)], ps_agg[ic][:])

            # transpose deg -> [128, 16] via DRAM bounce (keep this exact
            # structure: splitting it across rings or interleaving the
            # gather per-slice triggers an NRT exec-unit crash)
            deg_dram = dram_pool.tile([IC, 512], F32)
            for ic in range(IC):
                nc.scalar.dma_start(
                    deg_dram[ic : ic + 1, :], deg_sb[32 * ic : 32 * ic + 1, :]
                )
            nc.scalar.dma_start(
                degT[:], deg_dram.rearrange("a b -> (a b)").rearrange("(t p) -> p t", p=P)
            )
            nc.vector.reciprocal(rdeg[:], degT[:])

            # two output halves in separate tiles so the first half's DMA
            # leaves while the second half is still computing
            o_halves = [
                persist.tile([P, T // 2, D], F32, name=f"o_half{h}") for h in range(2)
            ]
            out_r = out_dram.rearrange("p (t d) -> p t d", t=T)
            with tc.tile_pool(name="fps", bufs=3, space="PSUM") as fps:
                for t in range(T):
                    h_ps = fps.tile([P, D], F32, tag="h")
                    nc.tensor.matmul(
                        h_ps[:], ag16[:, ts(t, P)], ut16[:], start=True, stop=True
                    )
                    o_dst = o_halves[t // (T // 2)][:, t % (T // 2), :]
                    if t % 2 == 0:
                        # ScalarE: out = relu(h * rdeg)
                        nc.scalar.activation(
                            o_dst,
                            h_ps[:],
                            mybir.ActivationFunctionType.Relu,
                            scale=rdeg[:, t : t + 1],
                        )
                    else:
                        # DVE: out = max(h * rdeg, 0)
                        nc.vector.tensor_scalar(
                            o_dst,
                            h_ps[:],
                            rdeg[:, t : t + 1],
                            0.0,
                            mybir.AluOpType.mult,
                            mybir.AluOpType.max,
                        )
                nc.scalar.dma_start(out_r[:, 0 : T // 2, :], o_halves[0][:])
                nc.sync.dma_start(out_r[:, T // 2 : T, :], o_halves[1][:])

    nc.compile()
    return nc


_NC = None


def _get_nc():
    global _NC
    if _NC is None:
        _NC = build_nc()
    return _NC


def prep_in_maps(x, adj_mat, U):
    import ml_dtypes

    x = np.asarray(x, dtype=np.float32)
    adj_mat = np.asarray(adj_mat)
    U = np.asarray(U, dtype=np.float32)
    # x -> fp16 [p, jb, d] flattened to [128, JB*D]
    xt = np.ascontiguousarray(
        x.reshape(JB, P, D).transpose(1, 0, 2).astype(np.float16).reshape(P, JB * D)
    )
    ut = np.ascontiguousarray(U.T.astype(np.float16))
    # adjacency values are {0,1}: exact in fp8e4m3, and the int8 bit
    # patterns 0x00/0x38 can be produced by a table lookup (much faster
    # than a float astype over 1 GiB)
    lut = np.zeros(2, dtype=np.uint8)
    lut[1] = np.array(1.0, dtype=ml_dtypes.float8_e4m3).view(np.uint8)
    in_maps = []
    for c in range(CORES):
        a8 = lut[adj_mat[:, c * S : (c + 1) * S]].view(ml_dtypes.float8_e4m3)
        in_maps.append({"a": a8, "x": xt, "ut": ut})
    return in_maps


def assemble_out(results):
    # per-core out is [128, T*D] in [i_lo, t, e] layout
    parts = []
    for c in range(CORES):
        o = results[c]["out"].reshape(P, T, D).transpose(1, 0, 2).reshape(S, D)
        parts.append(o)
    return np.concatenate(parts, axis=0)[None]


def kernel(x, adj_mat, U, **_):
    nc = _get_nc()
    in_maps = prep_in_maps(x, adj_mat, U)
    res = run_bass_kernel_spmd(nc, in_maps, core_ids=list(range(CORES)))
    return assemble_out(res.results)
